# revision 19
# baseline (speedup 1.0000x reference)
"""Trainium2 Bass kernel for the CinemaScalarImage SIREN/NeRF MLP.

Network (per point, N = 1048576 points, fp32):
  enc = [x, sin(2^k pi x), cos(2^k pi x)]  k=0..9            [N, 63]
  h = sin(30*(. @ W)) chain: enc->128->128->128->128 (W0..W3)
  x4 = h3 @ W4 + b4 [N,16]; density = relu(x4[:, 0]); scal = x4[:, 1:]
  s_in = [scal, enc] [N,78]; h5 = sin(30*(s_in@W5+b5)); h6 = sin(30*(h5@W6+b6))
  s = h6 @ W7 + b7 [N,1].  Returns (s, density).

Mapping to TRN2 (8 cores, pure data parallel over points):
  - feature-major layout: features on SBUF partitions, points on the free dim
  - all sine args computed in "turns" space y = arg/(2pi): weights pre-scaled
    by 30/(2pi) on host, so range reduction is y - round(y) (exact fp32),
    and the ACT Sin evaluates sin(2pi*d + bias) via its free fp32 affine
    (plain ACT Sin is only accurate to |arg| <~ pi, so EVERY sine needs this)
  - hidden layers: |y| <= 1.5 (host-checked ~1.3 max) so one ADD_RANGE_WRAP
    custom-DVE op wraps the psum into [-0.5, 0.5]
  - encoding: y = 2^(k-1)*x is exact; round(y+shift) via the +-1.5*2^23
    magic trick; two tile-pairs packed on partitions (rows 0:60 / 64:124)
    so each DVE op covers 2048 points; the 2-input subtract runs on GpSimd
  - matmuls float32r (1 cycle/col at free >= 256); rhs tiles [63, 1024]
    hold the 60 sine rows (ACT-written) plus raw x rows 60:63 (DMA-written)
    so L0/L5's encoder input is a single K=63 matmul per 512-col half
  - layer sines are one [128, 2048] ACT instr per layer per pair-group
  - W4[:,1:]@W5[:15] folded into one [128,128] matmul
"""
import sys

sys.path.insert(0, "/opt/trn_rl_repo")

import numpy as np
import concourse.bass as bass
import concourse.bacc as bacc
import concourse.tile as tile
from concourse import mybir
from concourse.bass_utils import run_bass_kernel_spmd

F32 = mybir.dt.float32
F32R = mybir.dt.float32r
AF = mybir.ActivationFunctionType
ALU = mybir.AluOpType

N = 1048576
NCORES = 8
NPC = N // NCORES          # 131072 points per core
T = 512                    # matmul free-dim (one PSUM bank at fp32)
TP = 1024                  # points per tile-pair (2 banks per psum)
GP = 2 * TP                # points per pair-group
GROUPS = NPC // GP         # 64
H = 128
ENC_SIN = 60
OMEGA = 30.0
TWO_PI = 2.0 * np.pi
MAGIC = float(np.float32(1.5 * 2 ** 23))
STAGE_GROUPS = 2           # groups per output staging buffer (4096 pts each)

_compiled = None


def _build():
    nc = bacc.Bacc("TRN2", target_bir_lowering=False)

    xT = nc.dram_tensor("xT", [3, NPC], F32, kind="ExternalInput")
    xTr = nc.dram_tensor("xTr", [3, NPC], F32R, kind="ExternalInput")
    wspec = {
        "w0sx": [63, H],
        "w1": [H, H], "w2": [H, H], "w3": [H, H],
        "w4c": [H, 1],
        "w45": [H, H], "w5sx": [63, H],
        "w6": [H, H], "w7": [H, 1],
    }
    wdram = {k: nc.dram_tensor(k, shp, F32R, kind="ExternalInput") for k, shp in wspec.items()}
    # [128,1] per-partition constants, enc pattern duplicated at rows 0:60 / 64:124
    encscale = nc.dram_tensor("encscale", [H, 1], F32, kind="ExternalInput")
    encshift = nc.dram_tensor("encshift", [H, 1], F32, kind="ExternalInput")
    encbias = nc.dram_tensor("encbias", [H, 1], F32, kind="ExternalInput")
    lbias = nc.dram_tensor("lbias", [H, 6], F32, kind="ExternalInput")
    scal2 = nc.dram_tensor("scal2", [1, 4], F32, kind="ExternalInput")  # [b4_0, b7_0, -, -]
    s_out = nc.dram_tensor("s_out", [1, NPC], F32, kind="ExternalOutput")
    d_out = nc.dram_tensor("d_out", [1, NPC], F32, kind="ExternalOutput")

    with tile.TileContext(nc) as tc:
        with (
            tc.tile_pool(name="wpool", bufs=1) as wpool,
            tc.tile_pool(name="inp", bufs=4) as inp,
            tc.tile_pool(name="encp", bufs=2) as encp,
            tc.tile_pool(name="rhsp", bufs=6) as rhsp,
            tc.tile_pool(name="hid", bufs=6) as hid,
            tc.tile_pool(name="wrp", bufs=4) as wrp,
            tc.tile_pool(name="stage", bufs=2) as stagep,
            tc.tile_pool(name="yps", bufs=4, space="PSUM") as yps,
        ):
            wt = {}
            for k, shp in wspec.items():
                wt[k] = wpool.tile(shp, F32R, name=f"wt_{k}")
                nc.sync.dma_start(out=wt[k], in_=wdram[k][:, :])
            esc = wpool.tile([H, 1], F32)
            nc.sync.dma_start(out=esc, in_=encscale[:, :])
            esh = wpool.tile([H, 1], F32)
            nc.sync.dma_start(out=esh, in_=encshift[:, :])
            ebi = wpool.tile([H, 1], F32)
            nc.sync.dma_start(out=ebi, in_=encbias[:, :])
            lbi = wpool.tile([H, 6], F32)
            nc.sync.dma_start(out=lbi, in_=lbias[:, :])
            sc2 = wpool.tile([1, 4], F32)
            nc.sync.dma_start(out=sc2, in_=scal2[:, :])

            def emit_enc(g):
                """Encoding for group g: broadcast DMA, turns-space range
                reduction (2 pairs packed on partitions 0:60 / 64:124), sine."""
                gcol = g * GP
                x_bc = inp.tile([H, TP], F32, name="x_bc")
                for p in range(2):
                    nc.sync.dma_start(
                        out=x_bc[p * 64:p * 64 + ENC_SIN, :],
                        in_=bass.AP(tensor=xT, offset=gcol + p * TP,
                                    ap=[[0, 20], [NPC, 3], [1, TP]]),
                    )
                y_e = encp.tile([H, TP], F32, name="y_e")
                nc.vector.tensor_scalar(out=y_e, in0=x_bc, scalar1=esc[:, 0:1],
                                        scalar2=None, op0=ALU.mult)
                t_e = encp.tile([H, TP], F32, name="t_e")
                nc.vector.tensor_scalar(out=t_e, in0=y_e, scalar1=esh[:, 0:1],
                                        scalar2=MAGIC, op0=ALU.add, op1=ALU.add)
                k_e = encp.tile([H, TP], F32, name="k_e")
                nc.vector.tensor_scalar(out=k_e, in0=t_e, scalar1=MAGIC,
                                        scalar2=None, op0=ALU.subtract)
                d_e = encp.tile([H, TP], F32, name="d_e")
                nc.gpsimd.tensor_tensor(out=d_e, in0=y_e, in1=k_e, op=ALU.subtract)

                # rhs tiles [63, TP]: sine rows 0:60 (ACT), raw x rows 60:63 (DMA)
                enc63 = []
                for p in range(2):
                    e63 = rhsp.tile([63, TP], F32R, name="e63")
                    off = p * 64
                    nc.scalar.activation(e63[0:ENC_SIN, :], d_e[off:off + ENC_SIN, :],
                                         AF.Sin, bias=ebi[off:off + ENC_SIN, 0:1],
                                         scale=TWO_PI)
                    nc.sync.dma_start(out=e63[ENC_SIN:63, :],
                                      in_=xTr[:, gcol + p * TP: gcol + (p + 1) * TP])
                    enc63.append(e63)
                return enc63

            stage_d = stage_s = None
            enc_next = emit_enc(0)
            for g in range(GROUPS):
                gcol = g * GP
                if g % STAGE_GROUPS == 0:
                    stage_d = stagep.tile([1, STAGE_GROUPS * GP], F32, name="stage_d")
                    stage_s = stagep.tile([1, STAGE_GROUPS * GP], F32, name="stage_s")
                enc63 = enc_next

                def layer(wk, rhs_pair, li, kdim=H, extra=None):
                    """per-pair: matmul -> psum [H,TP], wrap (DVE), sine (ACT)."""
                    out = []
                    for p in range(2):
                        yp = yps.tile([H, TP], F32, name="yp")
                        for c in (0, T):
                            nc.tensor.matmul(yp[:, c:c + T], wt[wk][0:kdim, :],
                                             rhs_pair[p][0:kdim, c:c + T],
                                             start=True, stop=extra is None)
                            if extra is not None:
                                ewk, erhs, ekdim = extra
                                nc.tensor.matmul(yp[:, c:c + T], wt[ewk][0:ekdim, :],
                                                 erhs[p][0:ekdim, c:c + T],
                                                 start=False, stop=True)
                        wr_ = wrp.tile([H, TP], F32, name="wr")
                        nc.vector.add_range_wrap(wr_, yp, 0.0, 0.5, 1.0)
                        hp = hid.tile([H, TP], F32R, name="hp")
                        nc.scalar.activation(hp, wr_, AF.Sin, bias=lbi[:, li:li + 1],
                                             scale=TWO_PI)
                        out.append(hp)
                    return out

                # ---- L0 (K=63 from enc63) then L1..L3
                h = layer("w0sx", enc63, 0, kdim=63)
                for li, wk in ((1, "w1"), (2, "w2"), (3, "w3")):
                    h = layer(wk, h, li)
                h3 = h

                # prefetch next group's encoding: overlaps this group's tail
                if g + 1 < GROUPS:
                    enc_next = emit_enc(g + 1)

                # ---- L5: W45 @ h3 + w5sx @ enc63 (emitted before the density
                # matmuls so the M=1 work stays off the critical PE path)
                h = layer("w45", h3, 4, extra=("w5sx", enc63, 63))

                # ---- density: relu(W4[:,0]^T h3 + b4_0) on ACT, per 512-col half
                for p in range(2):
                    scol = (g % STAGE_GROUPS) * GP + p * TP
                    for c in (0, T):
                        dp = yps.tile([1, T], F32, name="yp")
                        nc.tensor.matmul(dp, wt["w4c"], h3[p][:, c:c + T],
                                         start=True, stop=True)
                        nc.scalar.activation(stage_d[0:1, scol + c:scol + c + T], dp,
                                             AF.Relu, bias=sc2[0:1, 0:1])
                h6 = layer("w6", h, 5)

                # ---- L7: s = W7^T h6 + b7 on DVE, per 512-col half
                for p in range(2):
                    scol = (g % STAGE_GROUPS) * GP + p * TP
                    for c in (0, T):
                        sp = yps.tile([1, T], F32, name="yp")
                        nc.tensor.matmul(sp, wt["w7"], h6[p][:, c:c + T],
                                         start=True, stop=True)
                        nc.vector.tensor_scalar(out=stage_s[0:1, scol + c:scol + c + T],
                                                in0=sp, scalar1=sc2[0:1, 1:2],
                                                scalar2=None, op0=ALU.add)

                if (g + 1) % STAGE_GROUPS == 0:
                    base = (g + 1 - STAGE_GROUPS) * GP
                    w = STAGE_GROUPS * GP
                    nc.sync.dma_start(out=d_out[0:1, base:base + w], in_=stage_d[0:1, :])
                    nc.sync.dma_start(out=s_out[0:1, base:base + w], in_=stage_s[0:1, :])

    nc.compile()
    return nc


def _prep_host(inputs):
    """Host-side weight prep in float64, returns the per-core input maps."""
    f8 = {k: np.asarray(v, dtype=np.float64) for k, v in inputs.items()}
    W0, W1, W2, W3, W4, W5, W6, W7 = (f8[f"W{i}"] for i in range(8))
    b0, b1, b2, b3, b4, b5, b6, b7 = (f8[f"b{i}"] for i in range(8))
    SC = OMEGA / TWO_PI

    w = {}
    w["w0sx"] = np.concatenate([W0[3:63], W0[0:3]], axis=0) * SC
    w["w1"] = W1 * SC
    w["w2"] = W2 * SC
    w["w3"] = W3 * SC
    w["w4c"] = W4[:, 0:1]
    w["w45"] = (W4[:, 1:16] @ W5[0:15]) * SC
    w["w5sx"] = np.concatenate([W5[18:78], W5[15:18]], axis=0) * SC
    w["w6"] = W6 * SC
    w["w7"] = W7
    w = {k: np.ascontiguousarray(v, dtype=np.float32) for k, v in w.items()}

    # enc row constants; broadcast row = c*3+d (c-th copy, dim d):
    # freq k = c//2, sin for even c, cos for odd c. Duplicated at offset 64.
    escale = np.zeros((H, 1), np.float32)
    eshift = np.zeros((H, 1), np.float32)
    ebias = np.zeros((H, 1), np.float32)
    for c in range(20):
        k, is_cos = c // 2, c % 2
        for d in range(3):
            for off in (0, 64):
                r = off + c * 3 + d
                escale[r] = 2.0 ** (k - 1)
                if is_cos:
                    eshift[r] = 0.25
                    ebias[r] = np.pi / 2

    b5p = b5 + b4[1:16] @ W5[0:15]
    lb = np.zeros((H, 6), np.float32)
    for i, b in enumerate([b0, b1, b2, b3, b5p, b6]):
        lb[:, i] = (OMEGA * b).astype(np.float32)
    sc2 = np.array([[b4[0], b7[0], 0.0, 0.0]], np.float32)

    xT_full = np.ascontiguousarray(np.asarray(inputs["input_points"], np.float32).T)  # [3, N]
    in_maps = []
    for c in range(NCORES):
        m = {k: v for k, v in w.items()}
        m["encscale"] = escale
        m["encshift"] = eshift
        m["encbias"] = ebias
        m["lbias"] = lb
        m["scal2"] = sc2
        xc = np.ascontiguousarray(xT_full[:, c * NPC:(c + 1) * NPC])
        m["xT"] = xc
        m["xTr"] = xc
        in_maps.append(m)
    return in_maps


def kernel(**inputs):
    global _compiled
    if _compiled is None:
        _compiled = _build()
    nc = _compiled
    in_maps = _prep_host(inputs)
    res = run_bass_kernel_spmd(nc, in_maps, list(range(NCORES)))
    s = np.concatenate([r["s_out"].reshape(-1) for r in res.results]).reshape(N, 1)
    d = np.concatenate([r["d_out"].reshape(-1) for r in res.results]).reshape(N)
    return s.astype(np.float32), d.astype(np.float32)


# revision 20
# speedup vs baseline: 1.1613x; 1.1613x over previous
"""Trainium2 Bass kernel for the CinemaScalarImage SIREN/NeRF MLP.

Network (per point, N = 1048576 points, fp32):
  enc = [x, sin(2^k pi x), cos(2^k pi x)]  k=0..9            [N, 63]
  h = sin(30*(. @ W)) chain: enc->128->128->128->128 (W0..W3)
  x4 = h3 @ W4 + b4 [N,16]; density = relu(x4[:, 0]); scal = x4[:, 1:]
  s_in = [scal, enc] [N,78]; h5 = sin(30*(s_in@W5+b5)); h6 = sin(30*(h5@W6+b6))
  s = h6 @ W7 + b7 [N,1].  Returns (s, density).

Mapping to TRN2 (8 cores, pure data parallel over points):
  - feature-major layout: features on SBUF partitions, points on the free dim
  - all sine args computed in "turns" space y = arg/(2pi): weights pre-scaled
    by 30/(2pi) on host, so range reduction is y - round(y) (exact fp32),
    and the ACT Sin evaluates sin(2pi*d + bias) via its free fp32 affine
    (plain ACT Sin is only accurate to |arg| <~ pi, so EVERY sine needs this)
  - hidden layers: |y| <= 1.5 (host-checked ~1.3 max) so one ADD_RANGE_WRAP
    custom-DVE op wraps the psum into [-0.5, 0.5]
  - encoding: y = 2^(k-1)*x is exact; round(y+shift) via the +-1.5*2^23
    magic trick; two tile-pairs packed on partitions (rows 0:60 / 64:124)
    so each DVE op covers 2048 points; the 2-input subtract runs on GpSimd
  - matmuls float32r (1 cycle/col at free >= 256); rhs tiles [63, 1024]
    hold the 60 sine rows (ACT-written) plus raw x rows 60:63 (DMA-written)
    so L0/L5's encoder input is a single K=63 matmul per 512-col half
  - layer sines are one [128, 2048] ACT instr per layer per pair-group
  - W4[:,1:]@W5[:15] folded into one [128,128] matmul
"""
import sys

sys.path.insert(0, "/opt/trn_rl_repo")

import numpy as np
import concourse.bass as bass
import concourse.bacc as bacc
import concourse.tile as tile
from concourse import mybir
from concourse.bass_utils import run_bass_kernel_spmd

F32 = mybir.dt.float32
F32R = mybir.dt.float32r
AF = mybir.ActivationFunctionType
ALU = mybir.AluOpType

N = 1048576
NCORES = 8
NPC = N // NCORES          # 131072 points per core
T = 512                    # matmul free-dim (one PSUM bank at fp32)
TP = 1024                  # points per tile-pair (2 banks per psum)
GP = 2 * TP                # points per pair-group
GROUPS = NPC // GP         # 64
H = 128
ENC_SIN = 60
OMEGA = 30.0
TWO_PI = 2.0 * np.pi
MAGIC = float(np.float32(1.5 * 2 ** 23))
STAGE_GROUPS = 2           # groups per output staging buffer (4096 pts each)

_compiled = None


def _build():
    nc = bacc.Bacc("TRN2", target_bir_lowering=False)

    xT = nc.dram_tensor("xT", [3, NPC], F32, kind="ExternalInput")
    xTr = nc.dram_tensor("xTr", [3, NPC], F32R, kind="ExternalInput")
    wspec = {
        "w0sx": [63, H],
        "w1": [H, H], "w2": [H, H], "w3": [H, H],
        "w4c": [H, 1],
        "w45": [H, H], "w5sx": [63, H],
        "w6": [H, H], "w7": [H, 1],
    }
    wdram = {k: nc.dram_tensor(k, shp, F32R, kind="ExternalInput") for k, shp in wspec.items()}
    # [128,1] per-partition constants, enc pattern duplicated at rows 0:60 / 64:124
    encscale = nc.dram_tensor("encscale", [H, 1], F32, kind="ExternalInput")
    encshift = nc.dram_tensor("encshift", [H, 1], F32, kind="ExternalInput")
    encbias = nc.dram_tensor("encbias", [H, 1], F32, kind="ExternalInput")
    lbias = nc.dram_tensor("lbias", [H, 6], F32, kind="ExternalInput")
    scal2 = nc.dram_tensor("scal2", [1, 4], F32, kind="ExternalInput")  # [b4_0, b7_0, -, -]
    s_out = nc.dram_tensor("s_out", [1, NPC], F32, kind="ExternalOutput")
    d_out = nc.dram_tensor("d_out", [1, NPC], F32, kind="ExternalOutput")

    with tile.TileContext(nc) as tc:
        with (
            tc.tile_pool(name="wpool", bufs=1) as wpool,
            tc.tile_pool(name="inp", bufs=4) as inp,
            tc.tile_pool(name="encp", bufs=2) as encp,
            tc.tile_pool(name="rhsp", bufs=6) as rhsp,
            tc.tile_pool(name="hid", bufs=6) as hid,
            tc.tile_pool(name="wrp", bufs=4) as wrp,
            tc.tile_pool(name="stage", bufs=2) as stagep,
            tc.tile_pool(name="yps", bufs=3, space="PSUM") as yps,
            tc.tile_pool(name="sps", bufs=2, space="PSUM") as sps,
        ):
            wt = {}
            for k, shp in wspec.items():
                wt[k] = wpool.tile(shp, F32R, name=f"wt_{k}")
                nc.sync.dma_start(out=wt[k], in_=wdram[k][:, :])
            esc = wpool.tile([H, 1], F32)
            nc.sync.dma_start(out=esc, in_=encscale[:, :])
            esh = wpool.tile([H, 1], F32)
            nc.sync.dma_start(out=esh, in_=encshift[:, :])
            ebi = wpool.tile([H, 1], F32)
            nc.sync.dma_start(out=ebi, in_=encbias[:, :])
            lbi = wpool.tile([H, 6], F32)
            nc.sync.dma_start(out=lbi, in_=lbias[:, :])
            sc2 = wpool.tile([1, 4], F32)
            nc.sync.dma_start(out=sc2, in_=scal2[:, :])

            def emit_enc(g):
                """Encoding for group g: broadcast DMA, turns-space range
                reduction (2 pairs packed on partitions 0:60 / 64:124), sine."""
                gcol = g * GP
                x_bc = inp.tile([H, TP], F32, name="x_bc")
                for p in range(2):
                    nc.sync.dma_start(
                        out=x_bc[p * 64:p * 64 + ENC_SIN, :],
                        in_=bass.AP(tensor=xT, offset=gcol + p * TP,
                                    ap=[[0, 20], [NPC, 3], [1, TP]]),
                    )
                y_e = encp.tile([H, TP], F32, name="y_e")
                nc.vector.tensor_scalar(out=y_e, in0=x_bc, scalar1=esc[:, 0:1],
                                        scalar2=None, op0=ALU.mult)
                t_e = encp.tile([H, TP], F32, name="t_e")
                nc.vector.tensor_scalar(out=t_e, in0=y_e, scalar1=esh[:, 0:1],
                                        scalar2=MAGIC, op0=ALU.add, op1=ALU.add)
                k_e = encp.tile([H, TP], F32, name="k_e")
                nc.vector.tensor_scalar(out=k_e, in0=t_e, scalar1=MAGIC,
                                        scalar2=None, op0=ALU.subtract)
                d_e = encp.tile([H, TP], F32, name="d_e")
                nc.gpsimd.tensor_tensor(out=d_e, in0=y_e, in1=k_e, op=ALU.subtract)

                # rhs tiles [63, TP]: sine rows 0:60 (ACT), raw x rows 60:63 (DMA)
                enc63 = []
                for p in range(2):
                    e63 = rhsp.tile([63, TP], F32R, name="e63")
                    off = p * 64
                    nc.scalar.activation(e63[0:ENC_SIN, :], d_e[off:off + ENC_SIN, :],
                                         AF.Sin, bias=ebi[off:off + ENC_SIN, 0:1],
                                         scale=TWO_PI)
                    nc.sync.dma_start(out=e63[ENC_SIN:63, :],
                                      in_=xTr[:, gcol + p * TP: gcol + (p + 1) * TP])
                    enc63.append(e63)
                return enc63

            stage_d = stage_s = None
            enc_next = emit_enc(0)
            for g in range(GROUPS):
                gcol = g * GP
                if g % STAGE_GROUPS == 0:
                    stage_d = stagep.tile([1, STAGE_GROUPS * GP], F32, name="stage_d")
                    stage_s = stagep.tile([1, STAGE_GROUPS * GP], F32, name="stage_s")
                enc63 = enc_next

                def layer(wk, rhs_pair, li, kdim=H, extra=None):
                    """per-pair: matmul -> psum [H,TP], wrap (DVE), sine (ACT)."""
                    out = []
                    for p in range(2):
                        yp = yps.tile([H, TP], F32, name="yp")
                        for c in (0, T):
                            nc.tensor.matmul(yp[:, c:c + T], wt[wk][0:kdim, :],
                                             rhs_pair[p][0:kdim, c:c + T],
                                             start=True, stop=extra is None)
                            if extra is not None:
                                ewk, erhs, ekdim = extra
                                nc.tensor.matmul(yp[:, c:c + T], wt[ewk][0:ekdim, :],
                                                 erhs[p][0:ekdim, c:c + T],
                                                 start=False, stop=True)
                        wr_ = wrp.tile([H, TP], F32, name="wr")
                        nc.vector.add_range_wrap(wr_, yp, 0.0, 0.5, 1.0)
                        hp = hid.tile([H, TP], F32R, name="hp")
                        nc.scalar.activation(hp, wr_, AF.Sin, bias=lbi[:, li:li + 1],
                                             scale=TWO_PI)
                        out.append(hp)
                    return out

                # ---- L0 (K=63 from enc63) then L1..L3
                h = layer("w0sx", enc63, 0, kdim=63)
                for li, wk in ((1, "w1"), (2, "w2"), (3, "w3")):
                    h = layer(wk, h, li)
                h3 = h

                # prefetch next group's encoding: overlaps this group's tail
                if g + 1 < GROUPS:
                    enc_next = emit_enc(g + 1)

                # ---- L5: W45 @ h3 + w5sx @ enc63 (emitted before the density
                # matmuls so the M=1 work stays off the critical PE path)
                h = layer("w45", h3, 4, extra=("w5sx", enc63, 63))

                # ---- density: relu(W4[:,0]^T h3 + b4_0) on ACT, per 512-col half
                for p in range(2):
                    scol = (g % STAGE_GROUPS) * GP + p * TP
                    for c in (0, T):
                        dp = sps.tile([1, T], F32, name="dsp")
                        nc.tensor.matmul(dp, wt["w4c"], h3[p][:, c:c + T],
                                         start=True, stop=True)
                        nc.scalar.activation(stage_d[0:1, scol + c:scol + c + T], dp,
                                             AF.Relu, bias=sc2[0:1, 0:1])
                h6 = layer("w6", h, 5)

                # ---- L7: s = W7^T h6 + b7 on DVE, per 512-col half
                for p in range(2):
                    scol = (g % STAGE_GROUPS) * GP + p * TP
                    for c in (0, T):
                        sp = sps.tile([1, T], F32, name="dsp")
                        nc.tensor.matmul(sp, wt["w7"], h6[p][:, c:c + T],
                                         start=True, stop=True)
                        nc.vector.tensor_scalar(out=stage_s[0:1, scol + c:scol + c + T],
                                                in0=sp, scalar1=sc2[0:1, 1:2],
                                                scalar2=None, op0=ALU.add)

                if (g + 1) % STAGE_GROUPS == 0:
                    base = (g + 1 - STAGE_GROUPS) * GP
                    w = STAGE_GROUPS * GP
                    nc.sync.dma_start(out=d_out[0:1, base:base + w], in_=stage_d[0:1, :])
                    nc.sync.dma_start(out=s_out[0:1, base:base + w], in_=stage_s[0:1, :])

    nc.compile()
    return nc


def _prep_host(inputs):
    """Host-side weight prep in float64, returns the per-core input maps."""
    f8 = {k: np.asarray(v, dtype=np.float64) for k, v in inputs.items()}
    W0, W1, W2, W3, W4, W5, W6, W7 = (f8[f"W{i}"] for i in range(8))
    b0, b1, b2, b3, b4, b5, b6, b7 = (f8[f"b{i}"] for i in range(8))
    SC = OMEGA / TWO_PI

    w = {}
    w["w0sx"] = np.concatenate([W0[3:63], W0[0:3]], axis=0) * SC
    w["w1"] = W1 * SC
    w["w2"] = W2 * SC
    w["w3"] = W3 * SC
    w["w4c"] = W4[:, 0:1]
    w["w45"] = (W4[:, 1:16] @ W5[0:15]) * SC
    w["w5sx"] = np.concatenate([W5[18:78], W5[15:18]], axis=0) * SC
    w["w6"] = W6 * SC
    w["w7"] = W7
    w = {k: np.ascontiguousarray(v, dtype=np.float32) for k, v in w.items()}

    # enc row constants; broadcast row = c*3+d (c-th copy, dim d):
    # freq k = c//2, sin for even c, cos for odd c. Duplicated at offset 64.
    escale = np.zeros((H, 1), np.float32)
    eshift = np.zeros((H, 1), np.float32)
    ebias = np.zeros((H, 1), np.float32)
    for c in range(20):
        k, is_cos = c // 2, c % 2
        for d in range(3):
            for off in (0, 64):
                r = off + c * 3 + d
                escale[r] = 2.0 ** (k - 1)
                if is_cos:
                    eshift[r] = 0.25
                    ebias[r] = np.pi / 2

    b5p = b5 + b4[1:16] @ W5[0:15]
    lb = np.zeros((H, 6), np.float32)
    for i, b in enumerate([b0, b1, b2, b3, b5p, b6]):
        lb[:, i] = (OMEGA * b).astype(np.float32)
    sc2 = np.array([[b4[0], b7[0], 0.0, 0.0]], np.float32)

    xT_full = np.ascontiguousarray(np.asarray(inputs["input_points"], np.float32).T)  # [3, N]
    in_maps = []
    for c in range(NCORES):
        m = {k: v for k, v in w.items()}
        m["encscale"] = escale
        m["encshift"] = eshift
        m["encbias"] = ebias
        m["lbias"] = lb
        m["scal2"] = sc2
        xc = np.ascontiguousarray(xT_full[:, c * NPC:(c + 1) * NPC])
        m["xT"] = xc
        m["xTr"] = xc
        in_maps.append(m)
    return in_maps


def kernel(**inputs):
    global _compiled
    if _compiled is None:
        _compiled = _build()
    nc = _compiled
    in_maps = _prep_host(inputs)
    res = run_bass_kernel_spmd(nc, in_maps, list(range(NCORES)))
    s = np.concatenate([r["s_out"].reshape(-1) for r in res.results]).reshape(N, 1)
    d = np.concatenate([r["d_out"].reshape(-1) for r in res.results]).reshape(N)
    return s.astype(np.float32), d.astype(np.float32)


# revision 21
# speedup vs baseline: 1.2543x; 1.0800x over previous
"""Trainium2 Bass kernel for the CinemaScalarImage SIREN/NeRF MLP.

Network (per point, N = 1048576 points, fp32):
  enc = [x, sin(2^k pi x), cos(2^k pi x)]  k=0..9            [N, 63]
  h = sin(30*(. @ W)) chain: enc->128->128->128->128 (W0..W3)
  x4 = h3 @ W4 + b4 [N,16]; density = relu(x4[:, 0]); scal = x4[:, 1:]
  s_in = [scal, enc] [N,78]; h5 = sin(30*(s_in@W5+b5)); h6 = sin(30*(h5@W6+b6))
  s = h6 @ W7 + b7 [N,1].  Returns (s, density).

Mapping to TRN2 (8 cores, pure data parallel over points):
  - feature-major layout: features on SBUF partitions, points on the free dim
  - all sine args computed in "turns" space y = arg/(2pi): weights pre-scaled
    by 30/(2pi) on host, so range reduction is y - round(y) (exact fp32),
    and the ACT Sin evaluates sin(2pi*d + bias) via its free fp32 affine
    (plain ACT Sin is only accurate to |arg| <~ pi, so EVERY sine needs this)
  - hidden layers: |y| <= 1.5 (host-checked ~1.3 max) so one ADD_RANGE_WRAP
    custom-DVE op wraps the psum into [-0.5, 0.5]
  - encoding: y = 2^(k-1)*x is exact; round(y+shift) via the +-1.5*2^23
    magic trick; two tile-pairs packed on partitions (rows 0:60 / 64:124)
    so each DVE op covers 2048 points; the 2-input subtract runs on GpSimd
  - matmuls float32r (1 cycle/col at free >= 256); rhs tiles [63, 1024]
    hold the 60 sine rows (ACT-written) plus raw x rows 60:63 (DMA-written)
    so L0/L5's encoder input is a single K=63 matmul per 512-col half
  - layer sines are one [128, 2048] ACT instr per layer per pair-group
  - W4[:,1:]@W5[:15] folded into one [128,128] matmul
"""
import sys

sys.path.insert(0, "/opt/trn_rl_repo")

import numpy as np
import concourse.bass as bass
import concourse.bacc as bacc
import concourse.tile as tile
from concourse import mybir
from concourse.bass_utils import run_bass_kernel_spmd

F32 = mybir.dt.float32
F32R = mybir.dt.float32r
F16 = mybir.dt.float16
AF = mybir.ActivationFunctionType
ALU = mybir.AluOpType

N = 1048576
NCORES = 8
NPC = N // NCORES          # 131072 points per core
T = 512                    # matmul free-dim (one PSUM bank at fp32)
TP = 1024                  # points per tile-pair (2 banks per psum)
GP = 2 * TP                # points per pair-group
GROUPS = NPC // GP         # 64
H = 128
ENC_SIN = 60
OMEGA = 30.0
TWO_PI = 2.0 * np.pi
MAGIC = float(np.float32(1.5 * 2 ** 23))
STAGE_GROUPS = 2           # groups per output staging buffer (4096 pts each)

_compiled = None


def _build():
    nc = bacc.Bacc("TRN2", target_bir_lowering=False)

    xT = nc.dram_tensor("xT", [3, NPC], F32, kind="ExternalInput")
    xTr = nc.dram_tensor("xTr", [3, NPC], F16, kind="ExternalInput")
    wspec = {
        "w0sx": [63, H],
        "w1": [H, H], "w2": [H, H], "w3": [H, H],
        "w4c": [H, 1],
        "w45": [H, H], "w5sx": [63, H],
        "w6": [H, H], "w7": [H, 1],
    }
    wdram = {k: nc.dram_tensor(k, shp, F16, kind="ExternalInput") for k, shp in wspec.items()}
    # [128,1] per-partition constants, enc pattern duplicated at rows 0:60 / 64:124
    encscale = nc.dram_tensor("encscale", [H, 1], F32, kind="ExternalInput")
    encshift = nc.dram_tensor("encshift", [H, 1], F32, kind="ExternalInput")
    encbias = nc.dram_tensor("encbias", [H, 1], F32, kind="ExternalInput")
    lbias = nc.dram_tensor("lbias", [H, 6], F32, kind="ExternalInput")
    scal2 = nc.dram_tensor("scal2", [1, 4], F32, kind="ExternalInput")  # [b4_0, b7_0, -, -]
    s_out = nc.dram_tensor("s_out", [1, NPC], F32, kind="ExternalOutput")
    d_out = nc.dram_tensor("d_out", [1, NPC], F32, kind="ExternalOutput")

    with tile.TileContext(nc) as tc:
        with (
            tc.tile_pool(name="wpool", bufs=1) as wpool,
            tc.tile_pool(name="inp", bufs=4) as inp,
            tc.tile_pool(name="encp", bufs=2) as encp,
            tc.tile_pool(name="rhsp", bufs=6) as rhsp,
            tc.tile_pool(name="hid", bufs=6) as hid,
            tc.tile_pool(name="wrp", bufs=4) as wrp,
            tc.tile_pool(name="stage", bufs=2) as stagep,
            tc.tile_pool(name="yps", bufs=3, space="PSUM") as yps,
            tc.tile_pool(name="sps", bufs=2, space="PSUM") as sps,
        ):
            wt = {}
            for k, shp in wspec.items():
                wt[k] = wpool.tile(shp, F16, name=f"wt_{k}")
                nc.sync.dma_start(out=wt[k], in_=wdram[k][:, :])
            esc = wpool.tile([H, 1], F32)
            nc.sync.dma_start(out=esc, in_=encscale[:, :])
            esh = wpool.tile([H, 1], F32)
            nc.sync.dma_start(out=esh, in_=encshift[:, :])
            ebi = wpool.tile([H, 1], F32)
            nc.sync.dma_start(out=ebi, in_=encbias[:, :])
            lbi = wpool.tile([H, 6], F32)
            nc.sync.dma_start(out=lbi, in_=lbias[:, :])
            sc2 = wpool.tile([1, 4], F32)
            nc.sync.dma_start(out=sc2, in_=scal2[:, :])

            def emit_enc(g):
                """Encoding for group g: broadcast DMA, turns-space range
                reduction (2 pairs packed on partitions 0:60 / 64:124), sine."""
                gcol = g * GP
                x_bc = inp.tile([H, TP], F32, name="x_bc")
                for p in range(2):
                    nc.sync.dma_start(
                        out=x_bc[p * 64:p * 64 + ENC_SIN, :],
                        in_=bass.AP(tensor=xT, offset=gcol + p * TP,
                                    ap=[[0, 20], [NPC, 3], [1, TP]]),
                    )
                y_e = encp.tile([H, TP], F32, name="y_e")
                nc.vector.tensor_scalar(out=y_e, in0=x_bc, scalar1=esc[:, 0:1],
                                        scalar2=None, op0=ALU.mult)
                t_e = encp.tile([H, TP], F32, name="t_e")
                nc.vector.tensor_scalar(out=t_e, in0=y_e, scalar1=esh[:, 0:1],
                                        scalar2=MAGIC, op0=ALU.add, op1=ALU.add)
                k_e = encp.tile([H, TP], F32, name="k_e")
                nc.vector.tensor_scalar(out=k_e, in0=t_e, scalar1=MAGIC,
                                        scalar2=None, op0=ALU.subtract)
                d_e = encp.tile([H, TP], F32, name="d_e")
                nc.gpsimd.tensor_tensor(out=d_e, in0=y_e, in1=k_e, op=ALU.subtract)

                # rhs tiles [63, TP]: sine rows 0:60 (ACT), raw x rows 60:63 (DMA)
                enc63 = []
                for p in range(2):
                    e63 = rhsp.tile([63, TP], F16, name="e63")
                    off = p * 64
                    nc.scalar.activation(e63[0:ENC_SIN, :], d_e[off:off + ENC_SIN, :],
                                         AF.Sin, bias=ebi[off:off + ENC_SIN, 0:1],
                                         scale=TWO_PI)
                    nc.sync.dma_start(out=e63[ENC_SIN:63, :],
                                      in_=xTr[:, gcol + p * TP: gcol + (p + 1) * TP])
                    enc63.append(e63)
                return enc63

            stage_d = stage_s = None
            enc_next = emit_enc(0)
            for g in range(GROUPS):
                gcol = g * GP
                if g % STAGE_GROUPS == 0:
                    stage_d = stagep.tile([1, STAGE_GROUPS * GP], F32, name="stage_d")
                    stage_s = stagep.tile([1, STAGE_GROUPS * GP], F32, name="stage_s")
                enc63 = enc_next

                def layer(wk, rhs_pair, li, kdim=H, extra=None):
                    """per-pair: matmul -> psum [H,TP], wrap (DVE), sine (ACT)."""
                    out = []
                    for p in range(2):
                        yp = yps.tile([H, TP], F32, name="yp")
                        for c in (0, T):
                            nc.tensor.matmul(yp[:, c:c + T], wt[wk][0:kdim, :],
                                             rhs_pair[p][0:kdim, c:c + T],
                                             start=True, stop=extra is None)
                            if extra is not None:
                                ewk, erhs, ekdim = extra
                                nc.tensor.matmul(yp[:, c:c + T], wt[ewk][0:ekdim, :],
                                                 erhs[p][0:ekdim, c:c + T],
                                                 start=False, stop=True)
                        wr_ = wrp.tile([H, TP], F32, name="wr")
                        nc.vector.add_range_wrap(wr_, yp, 0.0, 0.5, 1.0)
                        hp = hid.tile([H, TP], F16, name="hp")
                        nc.scalar.activation(hp, wr_, AF.Sin, bias=lbi[:, li:li + 1],
                                             scale=TWO_PI)
                        out.append(hp)
                    return out

                # ---- L0 (K=63 from enc63) then L1..L3
                h = layer("w0sx", enc63, 0, kdim=63)
                for li, wk in ((1, "w1"), (2, "w2"), (3, "w3")):
                    h = layer(wk, h, li)
                h3 = h

                # prefetch next group's encoding: overlaps this group's tail
                if g + 1 < GROUPS:
                    enc_next = emit_enc(g + 1)

                # ---- L5: W45 @ h3 + w5sx @ enc63 (emitted before the density
                # matmuls so the M=1 work stays off the critical PE path)
                h = layer("w45", h3, 4, extra=("w5sx", enc63, 63))

                # ---- density: relu(W4[:,0]^T h3 + b4_0) on ACT, per 512-col half
                for p in range(2):
                    scol = (g % STAGE_GROUPS) * GP + p * TP
                    for c in (0, T):
                        dp = sps.tile([1, T], F32, name="dsp")
                        nc.tensor.matmul(dp, wt["w4c"], h3[p][:, c:c + T],
                                         start=True, stop=True)
                        nc.scalar.activation(stage_d[0:1, scol + c:scol + c + T], dp,
                                             AF.Relu, bias=sc2[0:1, 0:1])
                h6 = layer("w6", h, 5)

                # ---- L7: s = W7^T h6 + b7 on DVE, per 512-col half
                for p in range(2):
                    scol = (g % STAGE_GROUPS) * GP + p * TP
                    for c in (0, T):
                        sp = sps.tile([1, T], F32, name="dsp")
                        nc.tensor.matmul(sp, wt["w7"], h6[p][:, c:c + T],
                                         start=True, stop=True)
                        nc.vector.tensor_scalar(out=stage_s[0:1, scol + c:scol + c + T],
                                                in0=sp, scalar1=sc2[0:1, 1:2],
                                                scalar2=None, op0=ALU.add)

                if (g + 1) % STAGE_GROUPS == 0:
                    base = (g + 1 - STAGE_GROUPS) * GP
                    w = STAGE_GROUPS * GP
                    nc.sync.dma_start(out=d_out[0:1, base:base + w], in_=stage_d[0:1, :])
                    nc.sync.dma_start(out=s_out[0:1, base:base + w], in_=stage_s[0:1, :])

    nc.compile()
    return nc


def _prep_host(inputs):
    """Host-side weight prep in float64, returns the per-core input maps."""
    f8 = {k: np.asarray(v, dtype=np.float64) for k, v in inputs.items()}
    W0, W1, W2, W3, W4, W5, W6, W7 = (f8[f"W{i}"] for i in range(8))
    b0, b1, b2, b3, b4, b5, b6, b7 = (f8[f"b{i}"] for i in range(8))
    SC = OMEGA / TWO_PI

    w = {}
    w["w0sx"] = np.concatenate([W0[3:63], W0[0:3]], axis=0) * SC
    w["w1"] = W1 * SC
    w["w2"] = W2 * SC
    w["w3"] = W3 * SC
    w["w4c"] = W4[:, 0:1]
    w["w45"] = (W4[:, 1:16] @ W5[0:15]) * SC
    w["w5sx"] = np.concatenate([W5[18:78], W5[15:18]], axis=0) * SC
    w["w6"] = W6 * SC
    w["w7"] = W7
    w = {k: np.ascontiguousarray(v, dtype=np.float16) for k, v in w.items()}

    # enc row constants; broadcast row = c*3+d (c-th copy, dim d):
    # freq k = c//2, sin for even c, cos for odd c. Duplicated at offset 64.
    escale = np.zeros((H, 1), np.float32)
    eshift = np.zeros((H, 1), np.float32)
    ebias = np.zeros((H, 1), np.float32)
    for c in range(20):
        k, is_cos = c // 2, c % 2
        for d in range(3):
            for off in (0, 64):
                r = off + c * 3 + d
                escale[r] = 2.0 ** (k - 1)
                if is_cos:
                    eshift[r] = 0.25
                    ebias[r] = np.pi / 2

    b5p = b5 + b4[1:16] @ W5[0:15]
    lb = np.zeros((H, 6), np.float32)
    for i, b in enumerate([b0, b1, b2, b3, b5p, b6]):
        lb[:, i] = (OMEGA * b).astype(np.float32)
    sc2 = np.array([[b4[0], b7[0], 0.0, 0.0]], np.float32)

    xT_full = np.ascontiguousarray(np.asarray(inputs["input_points"], np.float32).T)  # [3, N]
    in_maps = []
    for c in range(NCORES):
        m = {k: v for k, v in w.items()}
        m["encscale"] = escale
        m["encshift"] = eshift
        m["encbias"] = ebias
        m["lbias"] = lb
        m["scal2"] = sc2
        xc = np.ascontiguousarray(xT_full[:, c * NPC:(c + 1) * NPC])
        m["xT"] = xc
        m["xTr"] = xc.astype(np.float16)
        in_maps.append(m)
    return in_maps


def kernel(**inputs):
    global _compiled
    if _compiled is None:
        _compiled = _build()
    nc = _compiled
    in_maps = _prep_host(inputs)
    res = run_bass_kernel_spmd(nc, in_maps, list(range(NCORES)))
    s = np.concatenate([r["s_out"].reshape(-1) for r in res.results]).reshape(N, 1)
    d = np.concatenate([r["d_out"].reshape(-1) for r in res.results]).reshape(N)
    return s.astype(np.float32), d.astype(np.float32)
